# revision 4
# baseline (speedup 1.0000x reference)
"""BitNet (ternary 2-bit packed) linear layer on 8 Trainium2 NeuronCores.

Problem: out[b,s,o] = (input @ unpack(weight_packed).T) * scale[o]
  input:         [4, 2048, 4096]  bf16
  weight_packed: [11008, 1024]    int8 (4 x 2-bit ternary codes per byte; 2 == -1)
  scale:         [11008, 1]       bf16
  out:           [4, 2048, 11008] bf16

Strategy (column-parallel, per sharding hint): shard out_features across the
8 cores (1376 rows each, padded to 1408 = 11*128), replicate the input.

Per-core kernel:
  - Host pre-transposes/permutes the replicated activations once:
      xt[512c + 128j + p, n] = x[n, 512c + 4p + j]
    so that the contraction dim lives on SBUF partitions with a permutation
    that matches how 2-bit fields unpack in-place (field j of byte b feeds
    In = 4b + j; both matmul operands use the same K-permutation, which is
    legal because the contraction is order-invariant).
  - Packed weight bytes are DMA'd in transposed layout [1024, 1408], unpacked
    on-chip by the vector engine into ternary bf16 K-tiles:
      w = ((u >> 2j) & 1) - ((u >> (2j+1)) & 1)   in {-1, 0, +1}
  - TensorE computes out[features, tokens] with weights stationary
    (128-feature x 128-K tiles) streaming 512-token chunks, fp32 PSUM
    accumulation over 32 K-tiles.
  - PSUM is rounded to bf16 (matching the reference's bf16 matmul output) and
    then scaled per-feature on the scalar engine, and written back as
    out_core[1408, 8192] bf16.
  - Host gathers the 8 shards, drops padding, transposes back.
"""

import numpy as np
import ml_dtypes

B, S = 4, 2048
IN_F, OUT_F = 4096, 11008
N_CORES = 8
TOKENS = B * S                    # 8192
F_PER_CORE = OUT_F // N_CORES     # 1376
F_TILES = (F_PER_CORE + 127) // 128   # 11
F_PAD = F_TILES * 128             # 1408
TCHUNK = 512

_CACHE = {}


def _build_program(in_f=IN_F, tokens=TOKENS, f_pad=F_PAD, tchunk=TCHUNK):
    import concourse.bass as bass
    import concourse.mybir as mybir
    import concourse.tile as tile
    from concourse import bacc

    kch = in_f // 512              # byte-chunks of 128 partitions
    ktiles = in_f // 128           # K tiles of 128
    ftiles = f_pad // 128
    nt = tokens // tchunk

    nc = bacc.Bacc(
        "TRN2", target_bir_lowering=False, debug=False, enable_asserts=False
    )
    xt = nc.dram_tensor(
        "xt_dram", [in_f, tokens], mybir.dt.bfloat16, kind="ExternalInput"
    ).ap()
    wp = nc.dram_tensor(
        "wp_dram", [in_f // 4, f_pad], mybir.dt.uint8, kind="ExternalInput"
    ).ap()
    sc = nc.dram_tensor(
        "sc_dram", [f_pad], mybir.dt.float32, kind="ExternalInput"
    ).ap()
    out = nc.dram_tensor(
        "out_dram", [f_pad, tokens], mybir.dt.bfloat16, kind="ExternalOutput"
    ).ap()

    AO = mybir.AluOpType
    with tile.TileContext(nc) as tc:
        with (
            tc.tile_pool(name="wpk", bufs=1) as wpk_pool,
            tc.tile_pool(name="wun", bufs=1) as wun_pool,
            tc.tile_pool(name="tmp", bufs=2) as tmp_pool,
            tc.tile_pool(name="xin", bufs=2) as x_pool,
            tc.tile_pool(name="oev", bufs=4) as o_pool,
            tc.tile_pool(name="ps", bufs=8, space="PSUM") as ps_pool,
        ):
            wp_sb = wpk_pool.tile([128, kch, f_pad], mybir.dt.uint8)
            nc.sync.dma_start(wp_sb[:], wp.rearrange("(c p) f -> p c f", p=128))
            sc_sb = wpk_pool.tile([128, ftiles], mybir.dt.float32)
            nc.sync.dma_start(sc_sb[:], sc.rearrange("(f p) -> p f", p=128))

            # Unpack ternary codes to bf16 K-tiles: kt = 4c + j.
            w_k = []
            for kt in range(ktiles):
                w_k.append(
                    wun_pool.tile(
                        [128, f_pad], mybir.dt.bfloat16,
                        name=f"w{kt}", tag=f"w{kt}",
                    )
                )
            for c in range(kch):
                u = wp_sb[:, c, :]
                for j in range(4):
                    a = tmp_pool.tile([128, f_pad], mybir.dt.uint8,
                                      name=f"a{c}_{j}", tag="a")
                    b = tmp_pool.tile([128, f_pad], mybir.dt.uint8,
                                      name=f"b{c}_{j}", tag="b")
                    nc.vector.tensor_scalar(
                        a[:], u, 2 * j, 1, AO.logical_shift_right, AO.bitwise_and
                    )
                    nc.vector.tensor_scalar(
                        b[:], u, 2 * j + 1, 1, AO.logical_shift_right, AO.bitwise_and
                    )
                    nc.vector.tensor_tensor(w_k[4 * c + j][:], a[:], b[:], AO.subtract)

            xt_r = xt.rearrange("(kt p) t -> p kt t", p=128)
            for t in range(nt):
                x_sb = x_pool.tile(
                    [128, ktiles, tchunk], mybir.dt.bfloat16,
                    name=f"x{t}", tag="x",
                )
                nc.sync.dma_start(
                    x_sb[:], xt_r[:, :, t * tchunk:(t + 1) * tchunk]
                )
                for f in range(ftiles):
                    ps = ps_pool.tile(
                        [128, tchunk], mybir.dt.float32,
                        name=f"ps{t}_{f}", tag="ps",
                    )
                    for kt in range(ktiles):
                        nc.tensor.matmul(
                            ps[:],
                            w_k[kt][:, f * 128:(f + 1) * 128],
                            x_sb[:, kt, :],
                            start=(kt == 0),
                            stop=(kt == ktiles - 1),
                        )
                    ob = o_pool.tile([128, tchunk], mybir.dt.bfloat16,
                                     name=f"ob{t}_{f}", tag="ob")
                    nc.vector.tensor_copy(ob[:], ps[:])
                    oc = o_pool.tile([128, tchunk], mybir.dt.bfloat16,
                                     name=f"oc{t}_{f}", tag="oc")
                    nc.scalar.mul(oc[:], ob[:], sc_sb[:, f:f + 1])
                    nc.sync.dma_start(
                        out[f * 128:(f + 1) * 128, t * tchunk:(t + 1) * tchunk],
                        oc[:],
                    )
    nc.compile()
    return nc


def _get_program():
    if "nc" not in _CACHE:
        _CACHE["nc"] = _build_program()
    return _CACHE["nc"]


def _prep_inputs(input, weight_packed, scale):
    """Host-side shard prep. Returns in_maps for the 8 cores."""
    x2 = np.ascontiguousarray(np.asarray(input)).reshape(TOKENS, IN_F)
    # xt[512c + 128j + p, n] = x2[n, 512c + 4p + j]
    xt = np.ascontiguousarray(
        x2.reshape(TOKENS, IN_F // 512, 128, 4)
        .transpose(1, 3, 2, 0)
        .reshape(IN_F, TOKENS)
    )
    wp = np.asarray(weight_packed)
    sc = np.asarray(scale)
    in_maps = []
    for i in range(N_CORES):
        wpad = np.zeros((F_PAD, IN_F // 4), np.int8)
        wpad[:F_PER_CORE] = wp[i * F_PER_CORE:(i + 1) * F_PER_CORE]
        wpT = np.ascontiguousarray(wpad.T).view(np.uint8)
        sci = np.zeros((F_PAD,), np.float32)
        sci[:F_PER_CORE] = sc[i * F_PER_CORE:(i + 1) * F_PER_CORE, 0].astype(np.float32)
        in_maps.append({"xt_dram": xt, "wp_dram": wpT, "sc_dram": sci})
    return in_maps


def _run(in_maps, trace=False, **kwargs):
    from concourse import bass_utils
    from concourse.bass_interp import get_hw_module

    nc = _get_program()
    old_m = nc.m
    nc.m = get_hw_module(nc.m)
    try:
        res = bass_utils.run_bass_kernel_spmd(
            nc, in_maps, core_ids=list(range(N_CORES)), trace=trace, **kwargs
        )
    finally:
        nc.m = old_m
    return res


def _gather(results):
    parts = [results[i]["out_dram"][:F_PER_CORE] for i in range(N_CORES)]
    big = np.concatenate(parts, axis=0)          # [11008, 8192] bf16
    return np.ascontiguousarray(big.T).reshape(B, S, OUT_F)


def kernel(input, weight_packed, scale):
    in_maps = _prep_inputs(input, weight_packed, scale)
    res = _run(in_maps, trace=False)
    return _gather(res.results)


# revision 7
# speedup vs baseline: 1.0070x; 1.0070x over previous
"""BitNet (ternary 2-bit packed) linear layer on 8 Trainium2 NeuronCores.

Problem: out[b,s,o] = (input @ unpack(weight_packed).T) * scale[o]
  input:         [4, 2048, 4096]  bf16
  weight_packed: [11008, 1024]    int8 (4 x 2-bit ternary codes per byte; 2 == -1)
  scale:         [11008, 1]       bf16
  out:           [4, 2048, 11008] bf16

Strategy (column-parallel, per sharding hint): shard out_features across the
8 cores (1376 rows each, padded to 1408 = 11*128), replicate the input.

Per-core kernel:
  - Host pre-transposes/permutes the replicated activations once:
      xt[512c + 128j + p, n] = x[n, 512c + 4p + j]
    so that the contraction dim lives on SBUF partitions with a permutation
    that matches how 2-bit fields unpack in-place (field j of byte b feeds
    In = 4b + j; both matmul operands use the same K-permutation, which is
    legal because the contraction is order-invariant).
  - Packed weight bytes are DMA'd in transposed layout [1024, 1408], unpacked
    on-chip by the vector engine into ternary bf16 K-tiles:
      w = ((u >> 2j) & 1) - ((u >> (2j+1)) & 1)   in {-1, 0, +1}
  - TensorE computes out[features, tokens] with weights stationary
    (128-feature x 128-K tiles) streaming 512-token chunks, fp32 PSUM
    accumulation over 32 K-tiles.
  - PSUM is rounded to bf16 (matching the reference's bf16 matmul output) and
    then scaled per-feature on the scalar engine, and written back as
    out_core[1408, 8192] bf16.
  - Host gathers the 8 shards, drops padding, transposes back.
"""

import numpy as np
import ml_dtypes

B, S = 4, 2048
IN_F, OUT_F = 4096, 11008
N_CORES = 8
TOKENS = B * S                    # 8192
F_PER_CORE = OUT_F // N_CORES     # 1376
F_TILES = (F_PER_CORE + 127) // 128   # 11
F_PAD = F_TILES * 128             # 1408
TCHUNK = 512

_CACHE = {}


def _build_program(in_f=IN_F, tokens=TOKENS, f_pad=F_PAD, tchunk=TCHUNK):
    import concourse.bass as bass
    import concourse.mybir as mybir
    import concourse.tile as tile
    from concourse import bacc

    kch = in_f // 512              # byte-chunks of 128 partitions
    ktiles = in_f // 128           # K tiles of 128
    ftiles = f_pad // 128
    nt = tokens // tchunk

    f4 = f_pad // 4                # packed bytes viewed as uint32 lanes
    MASK = 0x01010101              # bit0 of each byte in a u32 lane

    nc = bacc.Bacc(
        "TRN2", target_bir_lowering=False, debug=False, enable_asserts=False
    )
    xt = nc.dram_tensor(
        "xt_dram", [in_f, tokens], mybir.dt.bfloat16, kind="ExternalInput"
    ).ap()
    wp = nc.dram_tensor(
        "wp_dram", [in_f // 4, f4], mybir.dt.uint32, kind="ExternalInput"
    ).ap()
    sc = nc.dram_tensor(
        "sc_dram", [f_pad], mybir.dt.float32, kind="ExternalInput"
    ).ap()
    out = nc.dram_tensor(
        "out_dram", [f_pad, tokens], mybir.dt.bfloat16, kind="ExternalOutput"
    ).ap()

    AO = mybir.AluOpType
    with tile.TileContext(nc) as tc:
        with (
            tc.tile_pool(name="wpk", bufs=1) as wpk_pool,
            tc.tile_pool(name="wun", bufs=1) as wun_pool,
            tc.tile_pool(name="tmp", bufs=2) as tmp_pool,
            tc.tile_pool(name="xin", bufs=2) as x_pool,
            tc.tile_pool(name="oev", bufs=4) as o_pool,
            tc.tile_pool(name="ps", bufs=8, space="PSUM") as ps_pool,
        ):
            # Per-chunk packed-weight tiles (fine dependency granularity so
            # unpacking starts as soon as each chunk lands).
            wp_c = []
            for c in range(kch):
                wpc = wpk_pool.tile([128, f4], mybir.dt.uint32,
                                    name=f"wp{c}", tag=f"wp{c}")
                nc.sync.dma_start(wpc[:], wp[c * 128:(c + 1) * 128, :])
                wp_c.append(wpc)
            sc_sb = wpk_pool.tile([128, ftiles], mybir.dt.float32)
            nc.sync.dma_start(sc_sb[:], sc.rearrange("(f p) -> p f", p=128))

            # Unpack ternary codes to bf16 K-tiles: kt = 4c + j.
            # a = bit(2j), b = bit(2j+1) extracted 4-bytes-per-lane in u32,
            # then w = a - b (in {-1,0,1}) via byte-wise subtract to bf16,
            # alternating DVE / GPSIMD so the two engines pipeline.
            w_k = []
            for kt in range(ktiles):
                w_k.append(
                    wun_pool.tile(
                        [128, f_pad], mybir.dt.bfloat16,
                        name=f"w{kt}", tag=f"w{kt}",
                    )
                )
            for c in range(kch):
                u = wp_c[c][:]
                for j in range(4):
                    kt = 4 * c + j
                    a = tmp_pool.tile([128, f4], mybir.dt.uint32,
                                      name=f"a{c}_{j}", tag="a")
                    b = tmp_pool.tile([128, f4], mybir.dt.uint32,
                                      name=f"b{c}_{j}", tag="b")
                    nc.vector.tensor_scalar(
                        a[:], u, 2 * j, MASK, AO.logical_shift_right, AO.bitwise_and
                    )
                    nc.vector.tensor_scalar(
                        b[:], u, 2 * j + 1, MASK, AO.logical_shift_right, AO.bitwise_and
                    )
                    tt_eng = nc.vector if kt % 2 == 0 else nc.gpsimd
                    tt_eng.tensor_tensor(
                        w_k[kt][:],
                        a[:].bitcast(mybir.dt.uint8),
                        b[:].bitcast(mybir.dt.uint8),
                        AO.subtract,
                    )

            # Activation chunks stream on the ACT HWDGE ring (nc.scalar), in
            # 4 sub-tiles of 8 K-tiles each so the first matmuls start early;
            # weight/output DMAs use the SP ring (nc.sync).
            xt_r = xt.rearrange("(kt p) t -> p kt t", p=128)
            KSUB = min(8, ktiles)
            nparts = ktiles // KSUB
            for t in range(nt):
                x_parts = []
                for part in range(nparts):
                    xp = x_pool.tile(
                        [128, KSUB, tchunk], mybir.dt.bfloat16,
                        name=f"x{t}_{part}", tag=f"x{part}",
                    )
                    nc.scalar.dma_start(
                        xp[:],
                        xt_r[:, part * KSUB:(part + 1) * KSUB,
                             t * tchunk:(t + 1) * tchunk],
                    )
                    x_parts.append(xp)
                for f in range(ftiles):
                    ps = ps_pool.tile(
                        [128, tchunk], mybir.dt.float32,
                        name=f"ps{t}_{f}", tag="ps",
                    )
                    for kt in range(ktiles):
                        nc.tensor.matmul(
                            ps[:],
                            w_k[kt][:, f * 128:(f + 1) * 128],
                            x_parts[kt // KSUB][:, kt % KSUB, :],
                            start=(kt == 0),
                            stop=(kt == ktiles - 1),
                        )
                    ob = o_pool.tile([128, tchunk], mybir.dt.bfloat16,
                                     name=f"ob{t}_{f}", tag="ob")
                    nc.vector.tensor_copy(ob[:], ps[:])
                    oc = o_pool.tile([128, tchunk], mybir.dt.bfloat16,
                                     name=f"oc{t}_{f}", tag="oc")
                    nc.scalar.mul(oc[:], ob[:], sc_sb[:, f:f + 1])
                    nc.sync.dma_start(
                        out[f * 128:(f + 1) * 128, t * tchunk:(t + 1) * tchunk],
                        oc[:],
                    )
    nc.compile()
    return nc


def _get_program():
    if "nc" not in _CACHE:
        _CACHE["nc"] = _build_program()
    return _CACHE["nc"]


def _prep_inputs(input, weight_packed, scale):
    """Host-side shard prep. Returns in_maps for the 8 cores."""
    x2 = np.ascontiguousarray(np.asarray(input)).reshape(TOKENS, IN_F)
    # xt[512c + 128j + p, n] = x2[n, 512c + 4p + j]
    xt = np.ascontiguousarray(
        x2.reshape(TOKENS, IN_F // 512, 128, 4)
        .transpose(1, 3, 2, 0)
        .reshape(IN_F, TOKENS)
    )
    wp = np.asarray(weight_packed)
    sc = np.asarray(scale)
    in_maps = []
    for i in range(N_CORES):
        wpad = np.zeros((F_PAD, IN_F // 4), np.int8)
        wpad[:F_PER_CORE] = wp[i * F_PER_CORE:(i + 1) * F_PER_CORE]
        wpT = np.ascontiguousarray(wpad.T).view(np.uint32)
        sci = np.zeros((F_PAD,), np.float32)
        sci[:F_PER_CORE] = sc[i * F_PER_CORE:(i + 1) * F_PER_CORE, 0].astype(np.float32)
        in_maps.append({"xt_dram": xt, "wp_dram": wpT, "sc_dram": sci})
    return in_maps


def _run(in_maps, trace=False, **kwargs):
    from concourse import bass_utils
    from concourse.bass_interp import get_hw_module

    nc = _get_program()
    old_m = nc.m
    nc.m = get_hw_module(nc.m)
    try:
        res = bass_utils.run_bass_kernel_spmd(
            nc, in_maps, core_ids=list(range(N_CORES)), trace=trace, **kwargs
        )
    finally:
        nc.m = old_m
    return res


def _gather(results):
    parts = [results[i]["out_dram"][:F_PER_CORE] for i in range(N_CORES)]
    big = np.concatenate(parts, axis=0)          # [11008, 8192] bf16
    return np.ascontiguousarray(big.T).reshape(B, S, OUT_F)


def kernel(input, weight_packed, scale):
    in_maps = _prep_inputs(input, weight_packed, scale)
    res = _run(in_maps, trace=False)
    return _gather(res.results)


# revision 8
# speedup vs baseline: 1.0325x; 1.0253x over previous
"""BitNet (ternary 2-bit packed) linear layer on 8 Trainium2 NeuronCores.

Problem: out[b,s,o] = (input @ unpack(weight_packed).T) * scale[o]
  input:         [4, 2048, 4096]  bf16
  weight_packed: [11008, 1024]    int8 (4 x 2-bit ternary codes per byte; 2 == -1)
  scale:         [11008, 1]       bf16
  out:           [4, 2048, 11008] bf16

Strategy (column-parallel, per sharding hint): shard out_features across the
8 cores (1376 rows each, padded to 1408 = 11*128), replicate the input.

Per-core kernel:
  - Host pre-transposes/permutes the replicated activations once:
      xt[512c + 128j + p, n] = x[n, 512c + 4p + j]
    so that the contraction dim lives on SBUF partitions with a permutation
    that matches how 2-bit fields unpack in-place (field j of byte b feeds
    In = 4b + j; both matmul operands use the same K-permutation, which is
    legal because the contraction is order-invariant).
  - Packed weight bytes are DMA'd in transposed layout [1024, 1408], unpacked
    on-chip by the vector engine into ternary bf16 K-tiles:
      w = ((u >> 2j) & 1) - ((u >> (2j+1)) & 1)   in {-1, 0, +1}
  - TensorE computes out[features, tokens] with weights stationary
    (128-feature x 128-K tiles) streaming 512-token chunks, fp32 PSUM
    accumulation over 32 K-tiles.
  - PSUM is rounded to bf16 (matching the reference's bf16 matmul output) and
    then scaled per-feature on the scalar engine, and written back as
    out_core[1408, 8192] bf16.
  - Host gathers the 8 shards, drops padding, transposes back.
"""

import numpy as np
import ml_dtypes

B, S = 4, 2048
IN_F, OUT_F = 4096, 11008
N_CORES = 8
TOKENS = B * S                    # 8192
F_PER_CORE = OUT_F // N_CORES     # 1376
F_TILES = (F_PER_CORE + 127) // 128   # 11
F_PAD = F_TILES * 128             # 1408
TCHUNK = 512

_CACHE = {}


def _build_program(in_f=IN_F, tokens=TOKENS, f_pad=F_PAD, tchunk=TCHUNK):
    import concourse.bass as bass
    import concourse.mybir as mybir
    import concourse.tile as tile
    from concourse import bacc

    kch = in_f // 512              # byte-chunks of 128 partitions
    ktiles = in_f // 128           # K tiles of 128
    ftiles = f_pad // 128
    nt = tokens // tchunk

    f4 = f_pad // 4                # packed bytes viewed as uint32 lanes
    MASK = 0x01010101              # bit0 of each byte in a u32 lane

    nc = bacc.Bacc(
        "TRN2", target_bir_lowering=False, debug=False, enable_asserts=False
    )
    xt = nc.dram_tensor(
        "xt_dram", [in_f, tokens], mybir.dt.bfloat16, kind="ExternalInput"
    ).ap()
    wp = nc.dram_tensor(
        "wp_dram", [in_f // 4, f4], mybir.dt.uint32, kind="ExternalInput"
    ).ap()
    sc = nc.dram_tensor(
        "sc_dram", [f_pad], mybir.dt.float32, kind="ExternalInput"
    ).ap()
    out = nc.dram_tensor(
        "out_dram", [f_pad, tokens], mybir.dt.bfloat16, kind="ExternalOutput"
    ).ap()

    AO = mybir.AluOpType
    with tile.TileContext(nc) as tc:
        with (
            tc.tile_pool(name="wpk", bufs=1) as wpk_pool,
            tc.tile_pool(name="wun", bufs=1) as wun_pool,
            tc.tile_pool(name="tmp", bufs=2) as tmp_pool,
            tc.tile_pool(name="xin", bufs=2) as x_pool,
            tc.tile_pool(name="oev", bufs=4) as o_pool,
            tc.tile_pool(name="ps", bufs=8, space="PSUM") as ps_pool,
        ):
            # Per-chunk packed-weight tiles (fine dependency granularity so
            # unpacking starts as soon as each chunk lands).
            wp_c = []
            for c in range(kch):
                wpc = wpk_pool.tile([128, f4], mybir.dt.uint32,
                                    name=f"wp{c}", tag=f"wp{c}")
                nc.sync.dma_start(wpc[:], wp[c * 128:(c + 1) * 128, :])
                wp_c.append(wpc)
            sc_sb = wpk_pool.tile([128, ftiles], mybir.dt.float32)
            nc.sync.dma_start(sc_sb[:], sc.rearrange("(f p) -> p f", p=128))

            # Unpack ternary codes to bf16 K-tiles: kt = 4c + j.
            # a = bit(2j), b = bit(2j+1) extracted 4-bytes-per-lane in u32,
            # then w = a - b (in {-1,0,1}) via byte-wise subtract to bf16,
            # alternating DVE / GPSIMD so the two engines pipeline.
            w_k = []
            for kt in range(ktiles):
                w_k.append(
                    wun_pool.tile(
                        [128, f_pad], mybir.dt.bfloat16,
                        name=f"w{kt}", tag=f"w{kt}",
                    )
                )
            for c in range(kch):
                u = wp_c[c][:]
                for j in range(4):
                    kt = 4 * c + j
                    a = tmp_pool.tile([128, f4], mybir.dt.uint32,
                                      name=f"a{c}_{j}", tag="a")
                    b = tmp_pool.tile([128, f4], mybir.dt.uint32,
                                      name=f"b{c}_{j}", tag="b")
                    nc.vector.tensor_scalar(
                        a[:], u, 2 * j, MASK, AO.logical_shift_right, AO.bitwise_and
                    )
                    nc.vector.tensor_scalar(
                        b[:], u, 2 * j + 1, MASK, AO.logical_shift_right, AO.bitwise_and
                    )
                    nc.vector.tensor_tensor(
                        w_k[kt][:],
                        a[:].bitcast(mybir.dt.uint8),
                        b[:].bitcast(mybir.dt.uint8),
                        AO.subtract,
                    )

            # Activation chunks stream on the ACT HWDGE ring (nc.scalar), in
            # 4 sub-tiles of 8 K-tiles each so the first matmuls start early;
            # weight/output DMAs use the SP ring (nc.sync).
            xt_r = xt.rearrange("(kt p) t -> p kt t", p=128)
            KSUB = min(8, ktiles)
            nparts = ktiles // KSUB
            for t in range(nt):
                x_parts = []
                for part in range(nparts):
                    xp = x_pool.tile(
                        [128, KSUB, tchunk], mybir.dt.bfloat16,
                        name=f"x{t}_{part}", tag=f"x{part}",
                    )
                    nc.scalar.dma_start(
                        xp[:],
                        xt_r[:, part * KSUB:(part + 1) * KSUB,
                             t * tchunk:(t + 1) * tchunk],
                    )
                    x_parts.append(xp)
                for f in range(ftiles):
                    ps = ps_pool.tile(
                        [128, tchunk], mybir.dt.float32,
                        name=f"ps{t}_{f}", tag="ps",
                    )
                    for kt in range(ktiles):
                        nc.tensor.matmul(
                            ps[:],
                            w_k[kt][:, f * 128:(f + 1) * 128],
                            x_parts[kt // KSUB][:, kt % KSUB, :],
                            start=(kt == 0),
                            stop=(kt == ktiles - 1),
                        )
                    ob = o_pool.tile([128, tchunk], mybir.dt.bfloat16,
                                     name=f"ob{t}_{f}", tag="ob")
                    nc.vector.tensor_copy(ob[:], ps[:])
                    oc = o_pool.tile([128, tchunk], mybir.dt.bfloat16,
                                     name=f"oc{t}_{f}", tag="oc")
                    nc.scalar.mul(oc[:], ob[:], sc_sb[:, f:f + 1])
                    nc.sync.dma_start(
                        out[f * 128:(f + 1) * 128, t * tchunk:(t + 1) * tchunk],
                        oc[:],
                    )
    nc.compile()
    return nc


def _get_program():
    if "nc" not in _CACHE:
        _CACHE["nc"] = _build_program()
    return _CACHE["nc"]


def _prep_inputs(input, weight_packed, scale):
    """Host-side shard prep. Returns in_maps for the 8 cores."""
    x2 = np.ascontiguousarray(np.asarray(input)).reshape(TOKENS, IN_F)
    # xt[512c + 128j + p, n] = x2[n, 512c + 4p + j]
    xt = np.ascontiguousarray(
        x2.reshape(TOKENS, IN_F // 512, 128, 4)
        .transpose(1, 3, 2, 0)
        .reshape(IN_F, TOKENS)
    )
    wp = np.asarray(weight_packed)
    sc = np.asarray(scale)
    in_maps = []
    for i in range(N_CORES):
        wpad = np.zeros((F_PAD, IN_F // 4), np.int8)
        wpad[:F_PER_CORE] = wp[i * F_PER_CORE:(i + 1) * F_PER_CORE]
        wpT = np.ascontiguousarray(wpad.T).view(np.uint32)
        sci = np.zeros((F_PAD,), np.float32)
        sci[:F_PER_CORE] = sc[i * F_PER_CORE:(i + 1) * F_PER_CORE, 0].astype(np.float32)
        in_maps.append({"xt_dram": xt, "wp_dram": wpT, "sc_dram": sci})
    return in_maps


def _run(in_maps, trace=False, **kwargs):
    from concourse import bass_utils
    from concourse.bass_interp import get_hw_module

    nc = _get_program()
    old_m = nc.m
    nc.m = get_hw_module(nc.m)
    try:
        res = bass_utils.run_bass_kernel_spmd(
            nc, in_maps, core_ids=list(range(N_CORES)), trace=trace, **kwargs
        )
    finally:
        nc.m = old_m
    return res


def _gather(results):
    parts = [results[i]["out_dram"][:F_PER_CORE] for i in range(N_CORES)]
    big = np.concatenate(parts, axis=0)          # [11008, 8192] bf16
    return np.ascontiguousarray(big.T).reshape(B, S, OUT_F)


def kernel(input, weight_packed, scale):
    in_maps = _prep_inputs(input, weight_packed, scale)
    res = _run(in_maps, trace=False)
    return _gather(res.results)
